# revision 11
# baseline (speedup 1.0000x reference)
"""Trainium2 Bass kernel for nn_Net_74259984548321 (video-caption LSTM net).

Strategy: data-parallel over batch (8 rows/core, 8 cores), gates kept
TRANSPOSED (gate dim in partitions, batch in free dim) so the LSTM
elementwise runs on [128, 2, 8] tiles instead of [8, 512]:

  P1 : G1T = (feat @ e1_Wih.T + e1_b).T built in SBUF via bf16
       weight-stationary matmuls (two t-half passes, 8 PSUM banks each);
       cap_projT = (caption @ d2_Wih_l.T + d2_b).T likewise.
  P2 : 80-step 2-layer encoder LSTM, gates as [128 gpart, 8 gchunk, 8 b]
       PSUM tiles; per-step inputs injected via identity matmuls.
  P3 : 31-step decoder. Decoder cells use sigmoid(x) = (tanh(x/2)+1)/2
       (i,f,o weight rows pre-halved on host) so the whole decoder+P4
       phase stays on the tanh+exp activation table; decoder h states
       are stored doubled (h2x = 2h) with consumer weights pre-halved.
  P4 : output projection vs resident bf16 out_W (pre-halved), one-pass
       online sum(exp(logit)) + one-hot chunk-max target gather; log and
       the final CE reduction happen on the host.

All matmul operands are bf16 (fp32 PSUM accumulation); LSTM cell states
stay fp32 in SBUF.
"""

import numpy as np

B, T, FEAT, H, V, L = 64, 80, 4096, 256, 8000, 32
DEC = L - 1            # 31 decoder steps
NCORES = 8
BS = B // NCORES       # 8 batch rows per core
G = 4 * H              # 1024 gates
NCH = 16               # logit chunks
CSZ = V // NCH         # 500
ROWS = DEC * BS        # 248 (t, b) rows per core
KF = FEAT // 128       # 32 k-chunks of the feature dim
TB = T * BS            # 640 (t, b) encoder rows per core
TH = TB // 2           # 320: one t-half pass

_cache = {}


def _build_program():
    import concourse.tile as tile
    from concourse import bacc, mybir
    from concourse.bass import ts, ds
    from concourse.masks import make_identity

    fp = mybir.dt.float32
    bf = mybir.dt.bfloat16
    AF = mybir.ActivationFunctionType
    ALU = mybir.AluOpType
    AX = mybir.AxisListType

    nc = bacc.Bacc(None, target_bir_lowering=False)

    featT_d = nc.dram_tensor("featT", [128, KF, TB], bf, kind="ExternalInput")
    w1T_d = nc.dram_tensor("w1T", [128, KF, G], bf, kind="ExternalInput")
    b1T_d = nc.dram_tensor("b1T", [128, 8], fp, kind="ExternalInput")
    capT_d = nc.dram_tensor("capT", [128, 2, ROWS], bf, kind="ExternalInput")
    wd2lT_d = nc.dram_tensor("wd2lT", [128, 2, G], bf, kind="ExternalInput")
    bd2T_d = nc.dram_tensor("bd2T", [128, 8], fp, kind="ExternalInput")
    w1hhT_d = nc.dram_tensor("w1hhT", [128, 2, G], bf, kind="ExternalInput")
    w2T_d = nc.dram_tensor("w2T", [128, 4, G], bf, kind="ExternalInput")
    b2bc_d = nc.dram_tensor("b2bc", [128, 8, BS], bf, kind="ExternalInput")
    wd1T_d = nc.dram_tensor("wd1T", [128, 2, G], bf, kind="ExternalInput")
    bd1bc_d = nc.dram_tensor("bd1bc", [128, 8, BS], bf, kind="ExternalInput")
    wd2T_d = nc.dram_tensor("wd2T", [128, 4, G], bf, kind="ExternalInput")
    woT_d = nc.dram_tensor("woT", [128, 2, V], bf, kind="ExternalInput")
    bo_d = nc.dram_tensor("borow", [1, V], bf, kind="ExternalInput")
    oh_d = nc.dram_tensor("ohrows", [ROWS, V], fp, kind="ExternalInput")
    out_d = nc.dram_tensor("partial", [2, 128, 2], fp, kind="ExternalOutput")

    with tile.TileContext(nc) as tc:
        from contextlib import ExitStack

        with ExitStack() as ctx:
            const = ctx.enter_context(tc.tile_pool(name="const", bufs=1))
            wpool = ctx.enter_context(tc.tile_pool(name="w", bufs=1))
            state = ctx.enter_context(tc.tile_pool(name="state", bufs=1))
            hpool = ctx.enter_context(tc.tile_pool(name="hp", bufs=4))
            acts = ctx.enter_context(tc.tile_pool(name="acts", bufs=4))
            smsb = ctx.enter_context(tc.tile_pool(name="smsb", bufs=4))

            # ---- constants / identities / biases ----
            id128b = const.tile([128, 128], bf, tag="id128b")
            make_identity(nc, id128b)
            id8b = const.tile([BS, BS], bf, tag="id8b")
            make_identity(nc, id8b)
            ones1b = const.tile([1, 128], bf, tag="ones1b")
            nc.vector.memset(ones1b, 1.0)
            b1T = const.tile([128, 8], fp, tag="b1T")
            nc.sync.dma_start(b1T, b1T_d[:, :])
            bd2T = const.tile([128, 8], fp, tag="bd2T")
            nc.sync.dma_start(bd2T, bd2T_d[:, :])
            b2bc = const.tile([128, 8, BS], bf, tag="b2bc")
            nc.sync.dma_start(b2bc, b2bc_d[:, :, :])
            bd1bc = const.tile([128, 8, BS], bf, tag="bd1bc")
            nc.sync.dma_start(bd1bc, bd1bc_d[:, :, :])

            # ---- persistent weights (bf16) ----
            w1hh = wpool.tile([128, 2, G], bf, tag="w1hh")
            nc.sync.dma_start(w1hh, w1hhT_d[:, :, :])
            w2 = wpool.tile([128, 4, G], bf, tag="w2")
            nc.sync.dma_start(w2, w2T_d[:, :, :])
            wd1 = wpool.tile([128, 2, G], bf, tag="wd1")
            nc.sync.dma_start(wd1, wd1T_d[:, :, :])
            wd2 = wpool.tile([128, 4, G], bf, tag="wd2")
            nc.sync.dma_start(wd2, wd2T_d[:, :, :])
            wd2l = wpool.tile([128, 2, G], bf, tag="wd2l")
            nc.sync.dma_start(wd2l, wd2lT_d[:, :, :])
            # ---- persistent activations/state ----
            g1all = state.tile([128, 8, TB], bf, tag="g1all")
            capall = state.tile([128, 8, ROWS], bf, tag="capall")
            h2seqT = state.tile([128, 2, T, BS], bf, tag="h2seq")
            h2decT = state.tile([128, 2, DEC, BS], bf, tag="h2dec")
            A_sb = state.tile([T, BS, H], bf, tag="Asb")
            h2aT = state.tile([128, 2, BS], bf, tag="h2aT")
            # fused cell states: c12T[:, 0] = cell1's c, c12T[:, 1] = cell2's
            c12T = state.tile([128, 2, 2, BS], fp, tag="c12T")
            nc.vector.memset(c12T, 0.0)

            # ---- gate-PSUM pool first: lives through P1 + both loops ----
            gps = ctx.enter_context(tc.tile_pool(name="gps", bufs=4, space="PSUM"))

            # ================= P2 helpers (encoder) =================
            # Fused iteration: cell1[t+1] and cell2[t] share one gate PSUM
            # tile [128, 2cell, 8gc, 8b] so each activation / DVE op covers
            # both cells (per-op access latency dominates at this size).
            # Gate order [i, f, o, g]; per-cell chunk c: i=0:2 f=2:4 o=4:6 g=6:8.

            def cell1_mms(ps1, tn, h1prev):
                """cell1[tn] gate matmuls into ps1 [128, 8, BS]."""
                nc.tensor.matmul(
                    ps1, id128b, g1all[:, :, ds(tn * BS, BS)],
                    start=True, stop=False,
                )
                for gc in range(8):
                    for kc in range(2):
                        nc.tensor.matmul(
                            ps1[:, gc, :], w1hh[:, kc, ts(gc, 128)],
                            h1prev[:, kc, :], start=False, stop=(kc == 1),
                        )

            def lstm_elem2(ps, cT, houts):
                """Fused LSTM elementwise over n in {1,2} cells (sigmoid).

                ps/cT: [128, n, ...]; houts: list of (hout, idx)."""
                sifo = acts.tile(list(ps.shape[:2]) + [6, BS], fp, tag="sif")
                nc.scalar.activation(sifo, ps[:, :, 0:6, :], AF.Sigmoid)
                tg = acts.tile(list(ps.shape[:2]) + [2, BS], fp, tag="tg")
                nc.scalar.activation(tg, ps[:, :, 6:8, :], AF.Tanh)
                t1 = acts.tile(list(ps.shape[:2]) + [2, BS], fp, tag="t1")
                nc.vector.tensor_mul(t1, sifo[:, :, 0:2, :], tg)
                t2 = acts.tile(list(ps.shape[:2]) + [2, BS], fp, tag="t2")
                nc.vector.tensor_mul(t2, sifo[:, :, 2:4, :], cT)
                nc.vector.tensor_add(cT, t1, t2)
                th = acts.tile(list(ps.shape[:2]) + [2, BS], fp, tag="th")
                nc.scalar.activation(th, cT, AF.Tanh)
                for hout, i in houts:
                    nc.vector.tensor_mul(
                        hout, sifo[:, i, 4:6, :], th[:, i, :, :]
                    )

            enc_state = {}

            def enc_iter(t):
                h1cur = enc_state["h1"]
                last = t == T - 1
                ps = gps.tile([128, 2, 8, BS], fp, tag="g")
                if not last:
                    cell1_mms(ps[:, 0, :, :], t + 1, h1cur)
                # cell2[t]: e2_b + e2_Wih_r.T@h1[t] + e2_Whh.T@h2[t-1]
                nc.tensor.matmul(
                    ps[:, 1, :, :], id128b, b2bc, start=True, stop=False
                )
                nkc = 2 if t == 0 else 4
                for gc in range(8):
                    for kc in range(nkc):
                        lhs = (
                            h1cur[:, kc, :] if kc < 2
                            else h2seqT[:, kc - 2, t - 1, :]
                        )
                        nc.tensor.matmul(
                            ps[:, 1, gc, :], w2[:, kc, ts(gc, 128)], lhs,
                            start=False, stop=(kc == nkc - 1),
                        )
                if last:
                    lstm_elem2(
                        ps[:, 1:2, :, :], c12T[:, 1:2, :, :],
                        [(h2seqT[:, :, t, :], 0)],
                    )
                else:
                    h1nxt = hpool.tile([128, 2, BS], bf, tag="h1T")
                    lstm_elem2(
                        ps, c12T,
                        [(h1nxt, 0), (h2seqT[:, :, t, :], 1)],
                    )
                    enc_state["h1"] = h1nxt

            # ========== P1: G1T = (feat @ e1_Wih.T + e1_b).T ==========
            # 8 passes (t-half x gate-pair), 2 psum accumulators each.
            # Encoder steps 0..34 are emitted BETWEEN the two t-halves so
            # the chain-latency-bound encoder hides P1 PE work.
            ENC_SPLIT = 35
            with ExitStack() as p1ctx:
                fpool = p1ctx.enter_context(tc.tile_pool(name="fpool", bufs=1))
                w1s = p1ctx.enter_context(tc.tile_pool(name="w1s", bufs=2))
                p1ps = p1ctx.enter_context(
                    tc.tile_pool(name="p1ps", bufs=2, space="PSUM")
                )
                capps = p1ctx.enter_context(
                    tc.tile_pool(name="capps", bufs=1, space="PSUM")
                )
                # feat resident for P1 (scoped: freed for P4 buffers after);
                # chunked DMA so the first matmuls start early
                featall = fpool.tile([128, KF, TB], bf, tag="featall")
                for kq in range(4):
                    nc.sync.dma_start(
                        featall[:, ds(kq * 8, 8), :],
                        featT_d[:, ds(kq * 8, 8), :],
                    )
                # cap_projT (tiny; independent inputs, needed by decoder)
                capT = wpool.tile([128, 2, ROWS], bf, tag="capT")
                nc.sync.dma_start(capT, capT_d[:, :, :])
                for gc in range(8):
                    cps = capps.tile([128, ROWS], fp, tag="cp")
                    for kc in range(2):
                        nc.tensor.matmul(
                            cps, wd2l[:, kc, ts(gc, 128)], capT[:, kc, :],
                            start=(kc == 0), stop=(kc == 1),
                        )
                    nc.vector.tensor_scalar(
                        capall[:, gc, :], cps, bd2T[:, gc : gc + 1],
                        None, op0=ALU.add,
                    )

                KH = KF // 2

                def p1_pass(ph, gp):
                    accs = [
                        p1ps.tile([128, TH], fp, tag="fa",
                                  name=f"fa{ph}_{gp}_{gi}")
                        for gi in range(2)
                    ]
                    for kh in range(2):
                        w1t = w1s.tile([128, KH, 256], bf, tag="w1t")
                        nc.sync.dma_start(
                            w1t,
                            w1T_d[:, ds(kh * KH, KH), ds(gp * 256, 256)],
                        )
                        for k in range(KH):
                            for gi in range(2):
                                nc.tensor.matmul(
                                    accs[gi],
                                    w1t[:, k, ts(gi, 128)],
                                    featall[:, kh * KH + k, ds(ph * TH, TH)],
                                    start=(kh == 0 and k == 0),
                                    stop=(kh == 1 and k == KH - 1),
                                )
                    for gi in range(2):
                        gc = gp * 2 + gi
                        nc.vector.tensor_scalar(
                            g1all[:, gc, ds(ph * TH, TH)], accs[gi],
                            b1T[:, gc : gc + 1], None, op0=ALU.add,
                        )

                for gp in range(4):
                    p1_pass(0, gp)

                # ---- encoder prologue + first steps (need only t-half 0)
                # t = 0 cell1: gates come straight from G1T (h1 = 0)
                h1cur = hpool.tile([128, 2, BS], bf, tag="h1T")
                g10 = g1all[:, :, ds(0, BS)]
                sifo0 = acts.tile([128, 6, BS], fp, tag="sif")
                nc.scalar.activation(sifo0, g10[:, 0:6, :], AF.Sigmoid)
                tg0 = acts.tile([128, 2, BS], fp, tag="tg")
                nc.scalar.activation(tg0, g10[:, 6:8, :], AF.Tanh)
                nc.vector.tensor_mul(c12T[:, 0, :, :], sifo0[:, 0:2, :], tg0)
                th0 = acts.tile([128, 2, BS], fp, tag="th")
                nc.scalar.activation(th0, c12T[:, 0, :, :], AF.Tanh)
                nc.vector.tensor_mul(h1cur, sifo0[:, 4:6, :], th0)
                enc_state["h1"] = h1cur

                for t in range(ENC_SPLIT):
                    enc_iter(t)

                # ---- P1 second t-half: PE work rides under the encoder chain
                for gp in range(4):
                    p1_pass(1, gp)

            # ===== decoder-phase PSUM pools (reuse banks P1 released) =====
            p4ps = ctx.enter_context(tc.tile_pool(name="p4ps", bufs=1, space="PSUM"))
            smp = ctx.enter_context(tc.tile_pool(name="smp", bufs=2, space="PSUM"))
            ctp = ctx.enter_context(tc.tile_pool(name="ctp", bufs=1, space="PSUM"))

            for t in range(ENC_SPLIT, T):
                enc_iter(t)
                if t == 70:
                    # A_sb head (t_enc < 64) hides under the last enc steps
                    for b in range(BS):
                        for kc in range(2):
                            pA = smp.tile([64, 128], bf, tag="tr")
                            nc.tensor.transpose(
                                pA, h2seqT[:, kc, 0:64, b], id128b
                            )
                            nc.vector.tensor_copy(
                                A_sb[0:64, b, ts(kc, 128)], pA
                            )

            # out_W / out_b resident loads deferred here so P1's feat/w1
            # DMAs get the early bandwidth
            wo = wpool.tile([128, 2, V], bf, tag="wo")
            nc.sync.dma_start(wo, woT_d[:, :, :])
            bo_sb = wpool.tile([1, V], bf, tag="bo")
            nc.sync.dma_start(bo_sb, bo_d[:, :])

            # A_sb tail (t_enc 64..79); head was built under the encoder
            for b in range(BS):
                for kc in range(2):
                    pA = smp.tile([T, 128], bf, tag="tr")
                    nc.tensor.transpose(
                        pA[64:T], h2seqT[:, kc, 64:T, b], id128b
                    )
                    nc.vector.tensor_copy(A_sb[64:T, b, ts(kc, 128)], pA[64:T])

            # decoder boundary: h1 doubled, c states doubled (s = 2c)
            h1d = hpool.tile([128, 2, BS], bf, tag="h1T")
            nc.vector.tensor_scalar_mul(h1d, h1cur, 2.0)
            h1T = h1d
            nc.vector.tensor_scalar_mul(c12T, c12T, 2.0)

            # ---- P4 state ----
            s_all = state.tile([128, NCH, 2], fp, tag="s_all")
            mo_all = state.tile([128, NCH, 2], fp, tag="mo_all")
            tv_all = state.tile([128, NCH, 2], fp, tag="tv_all")

            ohs = ctx.enter_context(tc.tile_pool(name="ohs", bufs=2))
            junk = ctx.enter_context(tc.tile_pool(name="junk", bufs=2))

            def oh_load(mi):
                R = 128 if mi == 0 else ROWS - 128
                oht = ohs.tile([128, V], fp, tag="oh")
                nc.sync.dma_start(oht[:R], oh_d[ds(128 * mi, R), :])
                return oht

            def emit_p4(mi, oht):
                """logits + online exp-sum + one-hot chunk-max target gather."""
                R = 128 if mi == 0 else ROWS - 128
                tn = 16 if mi == 0 else DEC - 16
                for c in range(NCH):
                    psL = p4ps.tile([128, CSZ], fp, tag="psL")
                    nc.tensor.matmul(
                        psL[:R], ones1b[:, :R], bo_sb[:, ts(c, CSZ)],
                        start=True, stop=False,
                    )
                    for kc in range(2):
                        nc.tensor.matmul(
                            psL[:R],
                            h2decT[:, kc, ds(16 * mi, tn), :],
                            wo[:, kc, ts(c, CSZ)],
                            start=False, stop=(kc == 1),
                        )
                    ej = junk.tile([128, CSZ], fp, tag="jk")
                    nc.scalar.activation(
                        ej[:R], psL[:R], AF.Exp,
                        accum_out=s_all[:R, c, mi : mi + 1],
                    )
                    nc.vector.reduce_max(
                        mo_all[:R, c, mi : mi + 1], oht[:R, ts(c, CSZ)], axis=AX.X
                    )
                    tj = junk.tile([128, CSZ], fp, tag="jk")
                    nc.vector.scalar_tensor_tensor(
                        tj[:R], oht[:R, ts(c, CSZ)], mo_all[:R, c, mi : mi + 1], psL[:R],
                        op0=ALU.is_equal, op1=ALU.mult,
                        accum_out=tv_all[:R, c, mi : mi + 1],
                    )
                # combine: S = sum_c s_c ; tv = sum_c (mo_c == max_c mo_c) * tv_c
                st = smsb.tile([128, 2], fp, tag="st")
                nc.vector.reduce_sum(st[:R, 0:1], s_all[:R, :, mi], axis=AX.X)
                Moh = smsb.tile([128, 1], fp, tag="Moh")
                nc.vector.reduce_max(Moh[:R], mo_all[:R, :, mi], axis=AX.X)
                sj = smsb.tile([128, NCH], fp, tag="sj")
                nc.vector.scalar_tensor_tensor(
                    sj[:R], mo_all[:R, :, mi], Moh[:R], tv_all[:R, :, mi],
                    op0=ALU.is_equal, op1=ALU.mult,
                    accum_out=st[:R, 1:2],
                )
                nc.sync.dma_start(out_d[mi, ds(0, R), :], st[:R])

            # ================= P3: decoder =================
            # decoder sigmoid(x) = (tanh(x/2)+1)/2; i,f,o weight rows are
            # pre-halved on the host; h states stored doubled (h2x = 2h).
            def dec_elem2(ps, cT, houts):
                """Fused decoder elementwise over n cells (tanh-trick).

                s' = 0.5*(tf+1)*s + (ti+1)*tg ; h2x = (to+1)*tanh(s'/2)."""
                tifo = acts.tile(list(ps.shape[:2]) + [6, BS], fp, tag="sif")
                nc.scalar.activation(tifo, ps[:, :, 0:6, :], AF.Tanh)
                tg = acts.tile(list(ps.shape[:2]) + [2, BS], fp, tag="tg")
                nc.scalar.activation(tg, ps[:, :, 6:8, :], AF.Tanh)
                pp = acts.tile(list(ps.shape[:2]) + [2, BS], fp, tag="t1")
                nc.vector.scalar_tensor_tensor(
                    pp, tifo[:, :, 2:4, :], 1.0, cT, op0=ALU.add, op1=ALU.mult,
                )
                bb = acts.tile(list(ps.shape[:2]) + [2, BS], fp, tag="t2")
                nc.vector.scalar_tensor_tensor(
                    bb, tifo[:, :, 0:2, :], 1.0, tg, op0=ALU.add, op1=ALU.mult,
                )
                nc.vector.scalar_tensor_tensor(
                    cT, pp, 0.5, bb, op0=ALU.mult, op1=ALU.add,
                )
                th = acts.tile(list(ps.shape[:2]) + [2, BS], fp, tag="th")
                nc.scalar.activation(th, cT, AF.Tanh, scale=0.5)
                for hout, i in houts:
                    nc.vector.scalar_tensor_tensor(
                        hout, tifo[:, i, 4:6, :], 1.0, th[:, i, :, :],
                        op0=ALU.add, op1=ALU.mult,
                    )

            def d1_mms(ps1, h1prev):
                # d1: gates = bd1 + d1_Whh.T @ h1   (no input projection)
                nc.tensor.matmul(ps1, id128b, bd1bc, start=True, stop=False)
                for gc in range(8):
                    for kc in range(2):
                        nc.tensor.matmul(
                            ps1[:, gc, :], wd1[:, kc, ts(gc, 128)],
                            h1prev[:, kc, :], start=False, stop=(kc == 1),
                        )

            # d1[0] standalone (uses only the cell-0 half of a gate tile)
            ps0 = gps.tile([128, 2, 8, BS], fp, tag="g")
            d1_mms(ps0[:, 0, :, :], h1T)
            h1cur = hpool.tile([128, 2, BS], bf, tag="h1T")
            dec_elem2(ps0[:, 0:1, :, :], c12T[:, 0:1, :, :], [(h1cur, 0)])

            for t in range(DEC):
                last = t == DEC - 1
                ps = gps.tile([128, 2, 8, BS], fp, tag="g")
                if not last:
                    d1_mms(ps[:, 0, :, :], h1cur)
                # d2: gates = cap_projT[t] + d2_Wih_r.T@h1[t] + d2_Whh.T@h2
                nc.tensor.matmul(
                    ps[:, 1, :, :], id128b, capall[:, :, ds(t * BS, BS)],
                    start=True, stop=False,
                )
                for gc in range(8):
                    for kc in range(4):
                        if kc < 2:
                            lhs = h1cur[:, kc, :]
                        elif t == 0:
                            lhs = h2seqT[:, kc - 2, T - 1, :]
                        else:
                            lhs = h2aT[:, kc - 2, :]
                        nc.tensor.matmul(
                            ps[:, 1, gc, :], wd2[:, kc, ts(gc, 128)], lhs,
                            start=False, stop=(kc == 3),
                        )
                if last:
                    dec_elem2(
                        ps[:, 1:2, :, :], c12T[:, 1:2, :, :],
                        [(h2decT[:, :, t, :], 0)],
                    )
                else:
                    h1nxt = hpool.tile([128, 2, BS], bf, tag="h1T")
                    dec_elem2(
                        ps, c12T,
                        [(h1nxt, 0), (h2decT[:, :, t, :], 1)],
                    )
                # attention: scoresT [T, b] psum cols, then transpose to [8, 80]
                stps = smp.tile([T, BS], fp, tag="tr")
                for b in range(BS):
                    for kc in range(2):
                        nc.tensor.matmul(
                            stps[:, b : b + 1],
                            h2seqT[:, kc, :, b],
                            h2decT[:, kc, t, b : b + 1],
                            start=(kc == 0), stop=(kc == 1),
                        )
                sT_sb = acts.tile([T, BS], bf, tag="sT")
                nc.vector.tensor_copy(sT_sb, stps)
                scps = smp.tile([BS, T], bf, tag="tr")
                nc.tensor.transpose(scps, sT_sb, id128b[:T, :T])
                # softmax, 0.5 scale (scores are 2x: h2dec is doubled).
                # No max-subtraction: scores are dots of tanh-bounded states
                # (measured max |s| < 2), so exp cannot overflow.
                esb = acts.tile([BS, T], bf, tag="esb")
                sume = smsb.tile([BS, 1], fp, tag="sume")
                nc.scalar.activation(
                    esb, scps, AF.Exp, scale=0.5, accum_out=sume
                )
                recip = smsb.tile([BS, 1], fp, tag="rcp")
                nc.vector.reciprocal(recip, sume)
                attn = acts.tile([BS, T], bf, tag="attn")
                nc.vector.tensor_scalar_mul(attn, esb, recip)
                atps = smp.tile([T, BS], bf, tag="tr")
                nc.tensor.transpose(atps, attn, id8b)
                attnT = acts.tile([T, BS], bf, tag="attnT")
                nc.vector.tensor_copy(attnT, atps)
                ctps = ctp.tile([128, 2, BS], fp, tag="ctx")
                for b in range(BS):
                    for hc in range(2):
                        nc.tensor.matmul(
                            ctps[:, hc, b : b + 1],
                            A_sb[:, b, ts(hc, 128)],
                            attnT[:, b : b + 1],
                            start=True, stop=True,
                        )
                nc.vector.tensor_copy(h2aT, ctps)
                if not last:
                    h1cur = h1nxt
                # P4 epilogue once its rows are complete
                if t == 15:
                    oh0 = oh_load(0)
                    emit_p4(0, oh0)
                    oh1 = oh_load(1)    # prefetch under decoder t=16..30
                elif t == DEC - 1:
                    emit_p4(1, oh1)

    nc.compile()
    return nc


def _shard_inputs(inputs):
    """Host-side relayout + bf16 cast + shard. Returns list of 8 in_maps."""
    import ml_dtypes

    f32 = np.float32
    b16 = ml_dtypes.bfloat16
    feat = np.asarray(inputs["feat"], f32)
    caption = np.asarray(inputs["caption"], f32)
    oh = np.asarray(inputs["caption_one_hot"], f32)

    def w(name):
        return np.asarray(inputs[name], f32)

    # gate permutation: torch order [i, f, g, o] -> device order [i, f, o, g]
    # so the three sigmoid gates are contiguous (one activation op)
    perm = np.concatenate(
        [np.arange(0, 2 * H), np.arange(3 * H, G), np.arange(2 * H, 3 * H)]
    )

    # gate-column scale vectors in DEVICE order ([i, f, o] then [g])
    def gs(ifo, g):
        s = np.full(G, ifo, f32)
        s[3 * H :] = g
        return s

    s_half = gs(0.5, 1.0)      # sigma-trick only (true-scale operand)
    s_quarter = gs(0.25, 0.5)  # sigma-trick + doubled-h operand

    w1T = np.ascontiguousarray(
        w("e1_Wih").T[:, perm].reshape(KF, 128, G).transpose(1, 0, 2)
    ).astype(b16)
    b1T = np.ascontiguousarray(w("e1_b")[perm].reshape(8, 128).T).astype(f32)
    w1hhT = (
        w("e1_Whh").T[:, perm].reshape(2, 128, G).transpose(1, 0, 2).astype(b16)
    )
    w2T = (
        np.concatenate([w("e2_Wih")[:, H:], w("e2_Whh")], axis=1)
        .T[:, perm].reshape(4, 128, G).transpose(1, 0, 2).astype(b16)
    )
    b2bc = np.broadcast_to(
        w("e2_b")[perm].reshape(8, 128).T[:, :, None], (128, 8, BS)
    ).astype(b16)
    wd1T = (
        (w("d1_Whh").T[:, perm] * s_quarter[None, :])
        .reshape(2, 128, G).transpose(1, 0, 2).astype(b16)
    )
    bd1bc = np.broadcast_to(
        (w("d1_b")[perm] * s_half).reshape(8, 128).T[:, :, None], (128, 8, BS)
    ).astype(b16)
    wd2lT = (
        (w("d2_Wih")[:, :H].T[:, perm] * s_half[None, :])
        .reshape(2, 128, G).transpose(1, 0, 2).astype(b16)
    )
    bd2T = np.ascontiguousarray(
        (w("d2_b")[perm] * s_half).reshape(8, 128).T
    ).astype(f32)
    wd2T = (
        np.concatenate(
            [w("d2_Wih")[:, H:].T[:, perm] * s_quarter[None, :],
             w("d2_Whh").T[:, perm] * s_half[None, :]], axis=0,
        )
        .reshape(4, 128, G).transpose(1, 0, 2).astype(b16)
    )
    woT = (0.5 * w("out_W").T).reshape(2, 128, V).transpose(1, 0, 2).astype(b16)

    shared = dict(
        w1T=np.ascontiguousarray(w1T), b1T=b1T,
        w1hhT=np.ascontiguousarray(w1hhT),
        w2T=np.ascontiguousarray(w2T),
        b2bc=np.ascontiguousarray(b2bc),
        wd1T=np.ascontiguousarray(wd1T),
        bd1bc=np.ascontiguousarray(bd1bc),
        wd2lT=np.ascontiguousarray(wd2lT), bd2T=bd2T,
        wd2T=np.ascontiguousarray(wd2T),
        woT=np.ascontiguousarray(woT),
        borow=w("out_b").reshape(1, V).astype(b16),
    )

    in_maps = []
    for c in range(NCORES):
        b0 = c * BS
        featT = np.ascontiguousarray(
            feat[b0 : b0 + BS].transpose(2, 1, 0)
            .reshape(KF, 128, TB).transpose(1, 0, 2)
        ).astype(b16)
        capT = np.ascontiguousarray(
            caption[b0 : b0 + BS, : DEC]
            .transpose(2, 1, 0).reshape(2, 128, ROWS).transpose(1, 0, 2)
        ).astype(b16)
        ohrows = np.ascontiguousarray(
            oh[b0 : b0 + BS, 1:].transpose(1, 0, 2).reshape(ROWS, V)
        )
        m = dict(shared)
        m.update(featT=featT, capT=capT, ohrows=ohrows)
        in_maps.append(m)
    return in_maps


def kernel(**inputs):
    from concourse.bass_utils import run_bass_kernel_spmd

    if "nc" not in _cache:
        _cache["nc"] = _build_program()
    nc = _cache["nc"]
    in_maps = _shard_inputs(inputs)
    res = run_bass_kernel_spmd(nc, in_maps, core_ids=list(range(NCORES)))
    total = 0.0
    for r in res.results:
        part = np.asarray(r["partial"], np.float64)  # [2, 128, 2]
        for mi, R in ((0, 128), (1, ROWS - 128)):
            S = part[mi, :R, 0]
            tv = part[mi, :R, 1]
            total += float(np.sum(np.log(S) - tv))
    return np.asarray(total / (B * B), np.float32)
